# revision 22
# baseline (speedup 1.0000x reference)
"""Trainium2 Bass kernel for nn_LocalConnectivity (diamond-ring circular stencil).

out[i,j] = sum_{d=1..5} w_d * sum_{|di|+|dj|=d} x[(i+di)%H, (j+dj)%W]

Strategy: 4x2 grid shard across 8 NeuronCores (1024x2048 block each + 5-wide
circular halo prepped on host, all IO in bf16).  Per core, 9 row-windows of
M<=118 output rows (K=M+10 input rows).  Per window the diamond stencil is
split by |column shift| j:
  - VectorE folds the symmetric column pairs sigma_j = x(c-j)+x(c+j) for
    j=1..4 (bf16 tensor_add at 2x mode; odd-offset operands are skewed by one
    column so every operand is 4B-aligned, the skew is absorbed by the matmul
    rhs offset).
  - TensorE applies the vertical profiles as 7 banded bf16 matmuls per
    512-col chunk accumulating in one PSUM bank: the +-5 column shifts as two
    single-diagonal passes on x and V0 on x (11-tap band, center hole) go
    first (no sigma dependency), then V1..V4 on sigma_j (9/7/5/3-tap bands).
    bf16 streams ~1 cycle/col (vs ~2 for the old fp32r version).
  - ScalarE/VectorE alternate evicting PSUM->SBUF with the fp32->bf16 cast;
    each 512-col chunk is stored to HBM as soon as it is evicted.
Engine separation: strip loads + weights ride sync (HWDGE, which fans
HBM->SBUF loads across all 16 SDMA engines), stores ride gpsimd (SWDGE --
HWDGE stores collapse onto 2 SDMA engines), ACT only evicts, DVE only
folds.  A few dummy warm-up matmuls on a memset scratch tile keep the PE
HAM un-throttled through the initial DMA wait.
"""
import numpy as np
import ml_dtypes
from contextlib import ExitStack

import concourse.bass as bass
import concourse.tile as tile
from concourse import bacc, mybir
from concourse.bass_utils import run_bass_kernel_spmd

N_CORES = 8
H = W = 4096
MAXD = 5
GRID_R, GRID_C = 4, 2                 # core grid
BR, BC = H // GRID_R, W // GRID_C     # 1024 x 2048 block per core
IN_ROWS = BR + 2 * MAXD               # 1034
IN_COLS = BC + 2 * MAXD               # 2058
NCOL = 512                            # matmul free dim (one PSUM bank, fp32)
NCHUNK = BC // NCOL                   # 4
M_OUT = 118                           # out rows per window (K=128-2*MAXD)
NPASS = 7                             # V5-, V0, V5+, V1..V4(sigma)
N_WARM = 5                            # HAM warm-up matmuls
# windows: (out_row_start, M, K)
WINDOWS = []
_o = 0
while _o < BR:
    m = min(M_OUT, BR - _o)
    WINDOWS.append((_o, m, m + 2 * MAXD))
    _o += m
# sigma skew: built start cols (5-j-s, 5+j-s) are even => 4B-aligned bf16
SKEW = {j: (MAXD - j) % 2 for j in range(1, MAXD)}
SIGW = BC + 2                         # sigma tile free width (even)

_CACHE = {}


def _band_weights(distance_weights: np.ndarray) -> np.ndarray:
    """[128, 7*118] bf16 stationary band matrices W_p[k, m] (d = k-m-5).

    p=0: single diagonal w[5] at d==0 (column shift -5)
    p=1: V0 band  w[|d|] for 1<=|d|<=5
    p=2: single diagonal w[5] at d==0 (column shift +5)
    p=3..6: V_j band (j=1..4)  w[j+|d|] for |d|<=5-j
    """
    wd = np.asarray(distance_weights, dtype=np.float32)
    out = np.zeros((NPASS, 128, M_OUT), dtype=np.float32)
    k = np.arange(128)[:, None]
    m = np.arange(M_OUT)[None, :]
    d = k - m - MAXD
    ad = np.abs(d)
    diag = np.where(d == 0, wd[MAXD - 1], 0.0)
    out[0] = diag
    out[1] = np.where((ad >= 1) & (ad <= MAXD), wd[np.minimum(ad, MAXD) - 1], 0.0)
    out[2] = diag
    for j in range(1, MAXD):
        out[2 + j] = np.where(ad <= MAXD - j,
                              wd[np.minimum(j + ad, MAXD) - 1], 0.0)
    flat = np.ascontiguousarray(out.transpose(1, 0, 2).reshape(128, NPASS * M_OUT))
    return flat.astype(ml_dtypes.bfloat16)


def _build():
    dtb = mybir.dt.bfloat16
    dtf = mybir.dt.float32
    nc = bacc.Bacc("TRN2", target_bir_lowering=False, debug=False,
                   num_devices=N_CORES)
    x = nc.dram_tensor("x", [IN_ROWS, IN_COLS], dtb, kind="ExternalInput").ap()
    wts = nc.dram_tensor("w", [128, NPASS * M_OUT], dtb,
                         kind="ExternalInput").ap()
    y = nc.dram_tensor("y", [BR, BC], dtb, kind="ExternalOutput").ap()

    with tile.TileContext(nc) as tc, ExitStack() as ctx:
        spool = ctx.enter_context(tc.tile_pool(name="strip", bufs=4))
        gpool = ctx.enter_context(tc.tile_pool(name="sig", bufs=4))
        wpool = ctx.enter_context(tc.tile_pool(name="wts", bufs=1))
        opool = ctx.enter_context(tc.tile_pool(name="out", bufs=4))
        ppool = ctx.enter_context(tc.tile_pool(name="ps", bufs=8, space="PSUM"))

        # Weights first on the idle sync queue (they gate the first real MM),
        # then window 0's strip split across gpsimd + sync.
        wt = wpool.tile([128, NPASS * M_OUT], dtb)
        nc.sync.dma_start(wt[:], wts[:])
        CMID = IN_COLS // 2
        strips = {}
        o0, m0, k0 = WINDOWS[0]
        st = spool.tile([128, IN_COLS], dtb, tag="strip")
        nc.gpsimd.dma_start(st[:k0, :CMID], x[o0:o0 + k0, :CMID])
        nc.sync.dma_start(st[:k0, CMID:], x[o0:o0 + k0, CMID:])
        strips[0] = st
        # SBUF->HBM stores must ride SWDGE (gpsimd): HWDGE emits the
        # per-partition store descriptors onto only 2 of the 16 SDMA
        # engines (~50 GB/s); SWDGE's CounterMachine spreads all 16.

        # HAM warm-up: dummy matmuls on a zeroed scratch tile bridge the PE
        # from its preamble to the first data-gated matmul at full clock.
        zs = wpool.tile([128, M_OUT + NCOL], dtb, name="zs")
        nc.vector.memset(zs[:], 0)
        zp = ppool.tile([M_OUT, NCOL], dtf, tag="ps", name="zp")
        for _ in range(N_WARM):
            nc.tensor.matmul(zp[:], zs[:, :M_OUT], zs[:, M_OUT:],
                             start=True, stop=True)

        for wi, (out0, m, kdim) in enumerate(WINDOWS):
            if wi in strips:
                st = strips[wi]
            else:
                st = spool.tile([128, IN_COLS], dtb, tag="strip")
                nc.sync.dma_start(st[:kdim, :], x[out0:out0 + kdim, :])
            # sigma_j (j=1..4) on VectorE (one full-width 2x-mode add each;
            # window 0 builds in left/right halves so its first chunks'
            # matmuls are not gated on the full-width adds)
            sig = {}
            for j in range(1, MAXD):
                sig[j] = gpool.tile([128, SIGW], dtb, tag=f"sig{j}",
                                    name=f"sig{j}")
            halves = ((0, SIGW // 2 + 2), (SIGW // 2, SIGW)) if wi == 0 \
                else ((0, SIGW),)
            for h0, h1 in halves:
                for j in range(1, MAXD):
                    s = SKEW[j]
                    a0 = MAXD - j - s + h0
                    b0 = MAXD + j - s + h0
                    nc.vector.tensor_add(
                        sig[j][:kdim, h0:h1],
                        st[:kdim, a0:a0 + h1 - h0],
                        st[:kdim, b0:b0 + h1 - h0],
                    )
            # sigma_5 for the last chunk only: folds its two +-5 diagonal
            # passes into one (the diagonal band p=0 applies unchanged)
            c5 = (NCHUNK - 1) * NCOL
            sig5 = gpool.tile([128, NCOL], dtb, tag="sig5", name="sig5")
            nc.vector.tensor_add(
                sig5[:kdim, :],
                st[:kdim, c5:c5 + NCOL],
                st[:kdim, 2 * MAXD + c5:2 * MAXD + c5 + NCOL],
            )
            ot = opool.tile([m, BC], dtb, tag="out")
            for cc in range(NCHUNK):
                c0 = cc * NCOL
                ps = ppool.tile([m, NCOL], dtf, tag="ps")
                # strip-only passes first, then the sigma passes; the last
                # chunk uses sigma_5 in place of the two +-5 passes
                if cc == NCHUNK - 1:
                    plist = [0, 1, 3, 4, 5, 6]
                    rhs = [
                        sig5[:kdim, :],
                        st[:kdim, MAXD + c0:MAXD + c0 + NCOL],
                    ]
                else:
                    plist = list(range(NPASS))
                    rhs = [
                        st[:kdim, c0:c0 + NCOL],
                        st[:kdim, MAXD + c0:MAXD + c0 + NCOL],
                        st[:kdim, 2 * MAXD + c0:2 * MAXD + c0 + NCOL],
                    ]
                for j in range(1, MAXD):
                    s = SKEW[j]
                    rhs.append(sig[j][:kdim, s + c0:s + c0 + NCOL])
                for i, p in enumerate(plist):
                    nc.tensor.matmul(
                        ps[:],
                        wt[:kdim, p * M_OUT:p * M_OUT + m],
                        rhs[i],
                        start=(i == 0), stop=(i == len(plist) - 1),
                    )
                # Evict on ACT (DVE is busy folding sigmas), alternating
                # ACT/DVE only on the final window where DVE idles; store in
                # chunk pairs (2KB descriptor lines, half the completion
                # receipts) so the final stores' drain stays short.
                if wi == len(WINDOWS) - 1 and cc % 2 == 1:
                    nc.vector.tensor_copy(ot[:, c0:c0 + NCOL], ps[:])
                else:
                    nc.scalar.copy(ot[:, c0:c0 + NCOL], ps[:])
                if cc % 2 == 1:
                    nc.gpsimd.dma_start(
                        y[out0:out0 + m, c0 - NCOL:c0 + NCOL],
                        ot[:, c0 - NCOL:c0 + NCOL])
    nc.compile()
    return nc


def _make_in_maps(grid_spikes: np.ndarray, distance_weights: np.ndarray):
    x = np.ascontiguousarray(grid_spikes, dtype=np.float32)
    assert x.shape == (H, W)
    w_flat = _band_weights(distance_weights)
    xpad = np.pad(x, MAXD, mode="wrap").astype(ml_dtypes.bfloat16)
    in_maps = []
    for c in range(N_CORES):
        rb, cb = divmod(c, GRID_C)
        strip = xpad[rb * BR:rb * BR + IN_ROWS, cb * BC:cb * BC + IN_COLS]
        in_maps.append({"x": np.ascontiguousarray(strip), "w": w_flat})
    return in_maps


def kernel(grid_spikes: np.ndarray, distance_weights: np.ndarray) -> np.ndarray:
    if "nc" not in _CACHE:
        _CACHE["nc"] = _build()
    nc = _CACHE["nc"]

    in_maps = _make_in_maps(grid_spikes, distance_weights)
    res = run_bass_kernel_spmd(nc, in_maps, list(range(N_CORES)))
    out = np.empty((H, W), dtype=np.float32)
    for c in range(N_CORES):
        rb, cb = divmod(c, GRID_C)
        out[rb * BR:(rb + 1) * BR, cb * BC:(cb + 1) * BC] = \
            res.results[c]["y"].astype(np.float32)
    return out
